# revision 17
# baseline (speedup 1.0000x reference)
"""Trainium2 Bass kernel for nn_Decoder_3539053052044.

Structure (validated against the reference in numpy first):
- The reference decoder has a preserved bug: every layer consumes the ORIGINAL
  x0, so only the LAST layer's output survives. We compute layer L-1 only.
- Sequence-parallel: 8 cores x 256 tokens (core r -> batch r//4, chunk r%4).
  Each core computes the full last layer for its 256 tokens (K/V projections
  for its whole batch are computed locally), then one AllGather of y (bf16,
  0.5MB/rank), then a vocab-sharded projection (each core: all 2048 tokens x
  its 4000 vocab columns).
- Activations are feature-major [D on partitions, tokens free] so every linear
  layer uses the stored [D_in, D_out] weights directly as lhsT.
- Softmax is max-free (scores are O(1) for this model; exp(-1e9)=0 handles
  masking) and computed directly transposed, scoresT[k,q], so no transposes
  are needed; the per-(head,q) 1/sum is applied after the AV matmul via a
  K=1-matmul partition broadcast.
- LayerNorm runs feature-major via ones-matmul partition reductions.
- bf16 matmul inputs, fp32 accumulation (measured rel err ~3e-3 in the model).
"""

import numpy as np
import ml_dtypes

import concourse.bass as bass
import concourse.bacc as bacc
import concourse.tile as tile
from concourse import mybir
from concourse.bass_utils import run_bass_kernel_spmd
from concourse.vector_clock import ScopedClock, VectorClock

BF16 = ml_dtypes.bfloat16
F32 = mybir.dt.float32
BF = mybir.dt.bfloat16
PSUM = bass.MemorySpace.PSUM

B, S, D, H, L, V, DF = 2, 1024, 1024, 16, 4, 32000, 4096
DH = D // H              # 64
NC = 8                   # cores
TOK = B * S // NC        # 256 tokens per core
VS = V // NC             # 4000 vocab cols per core
KT = S // 128            # 8 k tiles
FT = D // 128            # 8 feature tiles
HT = DF // 128           # 32 hidden tiles
VN = 8                   # vocab n-chunks
VC = VS // VN            # 500 cols per chunk
ADD = mybir.AluOpType.add
MULT = mybir.AluOpType.mult
IDENT = mybir.ActivationFunctionType.Identity

_PATCHED = False


def _patch_tile_drain():
    """This neuronxcc build rejects a Drain carrying >1 sem wait. Split the
    Tile tail drain into one Drain per busy proc, each with a single wait."""
    global _PATCHED
    if _PATCHED:
        return
    _PATCHED = True

    def _drain_and_barrier_split(self, tick_clock, wait_clock):
        gc = tick_clock.global_clock
        n = len(gc)
        for p in range(n):
            if gc[p] > 0:
                vc = VectorClock([gc[q] if q == p else 0 for q in range(n)])
                d = self.nc.sync.drain()
                wait_clock.add_sem_waits(d.ins, ScopedClock({None: vc}))
        self.nc.sync.drain()
        self.nc.all_engine_barrier()
        assert self.sems is not None
        popped = self.nc._tile_sem_poison_stack.pop()
        assert popped is self._sem_poison
        self.nc.clear_and_free_semaphores(list(self.sems.allocated().values()))
        self.nc.all_engine_barrier()

    tile.TileContext._drain_and_barrier = _drain_and_barrier_split


def positional_encoding(seq_len, d_model, n=10000.0):
    i = np.arange(seq_len, dtype=np.float32)[:, None]
    d = np.arange(d_model)
    denom = np.power(n, (2 * (d // 2)).astype(np.float32) / d_model)
    ang = i / denom
    return np.where(d % 2 == 0, np.sin(ang), np.cos(ang)).astype(np.float32)


BIAS_NAMES = ['sbk', 'sbq', 'sbo', 'cbk', 'cbq', 'cbo', 'fb2',
              'ln1_g', 'ln1_b', 'ln2_g', 'ln2_b', 'ln3_g', 'ln3_b']


def build_program(self_mask_adds: bool, cross_mask_adds: bool):
    _patch_tile_drain()
    nc = bacc.Bacc()

    g = {}  # dram handles
    g['x0fm'] = nc.declare_dram_parameter("x0fm", [D, S], BF, isOutput=False)
    g['encfm'] = nc.declare_dram_parameter("encfm", [D, S], BF, isOutput=False)
    g['x0chunk'] = nc.declare_dram_parameter("x0chunk", [D, TOK], F32, isOutput=False)
    for w in ['sWq', 'sWk', 'sWv', 'sWo', 'cWq', 'cWk', 'cWv', 'cWo']:
        g[w] = nc.declare_dram_parameter(w, [D, D], BF, isOutput=False)
    g['fW1'] = nc.declare_dram_parameter("fW1", [D, DF], BF, isOutput=False)
    g['fW2'] = nc.declare_dram_parameter("fW2", [DF, D], BF, isOutput=False)
    g['Wout'] = nc.declare_dram_parameter("Wout", [D, VS], BF, isOutput=False)
    g['biases'] = nc.declare_dram_parameter("biases", [128, 8 * len(BIAS_NAMES)], F32, isOutput=False)
    g['fb1'] = nc.declare_dram_parameter("fb1", [128, HT], F32, isOutput=False)
    g['sbv_row'] = nc.declare_dram_parameter("sbv_row", [1, D], F32, isOutput=False)
    g['cbv_row'] = nc.declare_dram_parameter("cbv_row", [1, D], F32, isOutput=False)
    g['bout_row'] = nc.declare_dram_parameter("bout_row", [1, VS], F32, isOutput=False)
    g['maskT'] = nc.declare_dram_parameter("maskT", [S, TOK], F32, isOutput=False) if self_mask_adds else None
    g['maskTc'] = nc.declare_dram_parameter("maskTc", [S, TOK], F32, isOutput=False) if cross_mask_adds else None
    g['out'] = nc.declare_dram_parameter("out", [NC * TOK, VS], F32, isOutput=True)
    g['y_sh'] = nc.dram_tensor("y_sh", [D, TOK], BF)
    g['y_ag'] = nc.dram_tensor("y_ag", [NC, D, TOK], BF, addr_space="Shared")

    with tile.TileContext(nc) as tc:
        _emit(nc, tc, g)
    nc.compile()
    return nc


def _emit(nc, tc, g):
    from contextlib import ExitStack
    ctx = ExitStack()
    with ctx:
        # ---------- whole-kernel constants / small tensors ------------------
        const = ctx.enter_context(tc.tile_pool(name="const", bufs=1))
        ones_bf = const.tile([128, 1], BF, name="ones_bf", tag="c0")
        nc.gpsimd.memset(ones_bf[:], 1.0)
        ones_f32 = const.tile([128, 1], F32, name="ones_f32", tag="c1")
        nc.gpsimd.memset(ones_f32[:], 1.0)
        ones_row = const.tile([1, 128], F32, name="ones_row", tag="c2")
        nc.gpsimd.memset(ones_row[:], 1.0)
        bias_sb = const.tile([128, 8 * len(BIAS_NAMES)], F32, name="bias_sb", tag="c3")
        nc.sync.dma_start(bias_sb[:], g['biases'][:])
        fb1_sb = const.tile([128, HT], F32, name="fb1_sb", tag="c4")
        nc.sync.dma_start(fb1_sb[:], g['fb1'][:])
        def bias_col(name, f):
            i = BIAS_NAMES.index(name)
            return bias_sb[:, i * 8 + f:i * 8 + f + 1]

        # free-axis bias broadcast tiles [128, D] for sbv / cbv
        free_bias = {}
        with tc.tile_pool(name="bbc_ps", bufs=1, space=PSUM) as bps, \
             tc.tile_pool(name="bbc_row", bufs=2) as brow:
            for bi, bname in enumerate(['sbv', 'cbv']):
                t = const.tile([128, D], F32, name=f"{bname}_b", tag=f"fb{bi}")
                rsb = brow.tile([1, D], F32, tag="row")
                nc.sync.dma_start(rsb[:], g[f'{bname}_row'][:])
                for half in range(2):
                    ps = bps.tile([128, 512], F32, tag="bc")
                    nc.tensor.matmul(ps[:], ones_row[:],
                                     rsb[0:1, half * 512:(half + 1) * 512],
                                     start=True, stop=True)
                    nc.vector.tensor_copy(t[:, half * 512:(half + 1) * 512], ps[:])
                free_bias[bname] = t

        # LN outputs (outer lifetime)
        a1pool = ctx.enter_context(tc.tile_pool(name="a1", bufs=FT))
        a2pool = ctx.enter_context(tc.tile_pool(name="a2", bufs=FT))
        ypool = ctx.enter_context(tc.tile_pool(name="y", bufs=FT))

        # ---------- helpers -------------------------------------------------
        def proj_fm(w_name, act_tiles, n_tok, bias_name, out_pool, scale=None):
            """feature-major out tiles [FT x [128, n_tok]] bf16 = W.T @ act + b"""
            outs = []
            nsub = (n_tok + 511) // 512
            with tc.tile_pool(name=f"w_{w_name}", bufs=1) as wp, \
                 tc.tile_pool(name=f"ps_{w_name}", bufs=3, space=PSUM) as pp:
                wtiles = []
                for k in range(FT):
                    wt = wp.tile([128, D], BF, name=f"w_{w_name}_{k}", tag=f"k{k}")
                    nc.sync.dma_start(wt[:], g[w_name][k * 128:(k + 1) * 128, :])
                    wtiles.append(wt)
                for m in range(FT):
                    ot = out_pool.tile([128, n_tok], BF, name=f"o_{w_name}_{m}",
                                       tag=f"o_{w_name}_{m}", bufs=1)
                    for ns in range(nsub):
                        c0, c1 = ns * 512, min((ns + 1) * 512, n_tok)
                        ps = pp.tile([128, c1 - c0], F32, tag="ps")
                        for k in range(FT):
                            nc.tensor.matmul(ps[:], wtiles[k][:, m * 128:(m + 1) * 128],
                                             act_tiles[k][:, c0:c1],
                                             start=(k == 0), stop=(k == FT - 1))
                        if scale is not None:
                            nc.vector.tensor_scalar(ot[:, c0:c1], ps[:], scale,
                                                    bias_col(bias_name, m), MULT, ADD)
                        else:
                            nc.scalar.activation(ot[:, c0:c1], ps[:], IDENT,
                                                 bias=bias_col(bias_name, m))
                    outs.append(ot)
            return outs

        def proj_tm(w_name, act_tiles, bias_bcast, out_pool):
            """token-major out tiles [KT x [128, D]] bf16 = act.T @ W + b"""
            outs = []
            with tc.tile_pool(name=f"w_{w_name}", bufs=1) as wp, \
                 tc.tile_pool(name=f"ps_{w_name}", bufs=3, space=PSUM) as pp:
                wtiles = []
                for k in range(FT):
                    wt = wp.tile([128, D], BF, name=f"w_{w_name}_{k}", tag=f"k{k}")
                    nc.sync.dma_start(wt[:], g[w_name][k * 128:(k + 1) * 128, :])
                    wtiles.append(wt)
                for m in range(KT):
                    ot = out_pool.tile([128, D], BF, name=f"o_{w_name}_{m}",
                                       tag=f"o_{w_name}_{m}", bufs=1)
                    for ns in range(2):
                        c0, c1 = ns * 512, (ns + 1) * 512
                        ps = pp.tile([128, 512], F32, tag="ps")
                        for k in range(FT):
                            nc.tensor.matmul(ps[:], act_tiles[k][:, m * 128:(m + 1) * 128],
                                             wtiles[k][:, c0:c1],
                                             start=(k == 0), stop=(k == FT - 1))
                        nc.vector.scalar_tensor_tensor(ot[:, c0:c1], ps[:], 1.0,
                                                       bias_bcast[:, c0:c1], MULT, ADD)
                    outs.append(ot)
            return outs

        def attention(q_pairs, k_tiles, v_tiles, mask_tiles, tag, out_pool):
            """q_pairs feature-major [FT x [128,TOK]] bf16; k_tiles [FT x [128,S]];
            v_tiles token-major [KT x [128,D]]; returns [FT x [128,TOK]] bf16."""
            outs = []
            with tc.tile_pool(name=f"exp_{tag}", bufs=3) as epool, \
                 tc.tile_pool(name=f"asm_{tag}", bufs=4) as spool, \
                 tc.tile_pool(name=f"sT_{tag}", bufs=2, space=PSUM) as sps, \
                 tc.tile_pool(name=f"sum_{tag}", bufs=2, space=PSUM) as sums, \
                 tc.tile_pool(name=f"av_{tag}", bufs=2, space=PSUM) as avs, \
                 tc.tile_pool(name=f"rb_{tag}", bufs=2, space=PSUM) as rbs:
                for hp in range(FT):
                    at = out_pool.tile([128, TOK], BF, name=f"at_{tag}_{hp}",
                                       tag=f"at_{hp}", bufs=1)
                    for hh in range(2):
                        h = 2 * hp + hh
                        psum = sums.tile([1, TOK], F32, tag="sum")
                        av = avs.tile([64, TOK], F32, tag="av")
                        po = (h % 2) * 64
                        for kt in range(KT):
                            sT = sps.tile([128, TOK], F32, tag="sT")
                            nc.tensor.matmul(
                                sT[:],
                                k_tiles[hp][po:po + 64, kt * 128:(kt + 1) * 128],
                                q_pairs[hp][po:po + 64, :], start=True, stop=True)
                            if mask_tiles is not None:
                                nc.vector.tensor_add(sT[:], sT[:], mask_tiles[kt][:])
                            ex = epool.tile([128, TOK], BF, tag="exp")
                            nc.scalar.activation(ex[:], sT[:], mybir.ActivationFunctionType.Exp)
                            nc.tensor.matmul(psum[:], ones_bf[:], ex[:],
                                             start=(kt == 0), stop=(kt == KT - 1))
                            nc.tensor.matmul(av[:], v_tiles[kt][:, h * 64:(h + 1) * 64], ex[:],
                                             start=(kt == 0), stop=(kt == KT - 1))
                        rec = spool.tile([1, TOK], F32, tag="recip")
                        nc.vector.reciprocal(rec[:], psum[:])
                        rb = rbs.tile([64, TOK], F32, tag="rb")
                        nc.tensor.matmul(rb[:], ones_row[0:1, 0:64], rec[:], start=True, stop=True)
                        rb_sb = spool.tile([64, TOK], F32, tag="rb_sb")
                        nc.vector.tensor_copy(rb_sb[:], rb[:])
                        nc.vector.tensor_mul(at[hh * 64:hh * 64 + 64, :], av[:], rb_sb[:])
                    outs.append(at)
            return outs

        def o_proj_residual(w_name, attn_tiles, bo_name, resid_tiles, rpool):
            outs = []
            with tc.tile_pool(name=f"w_{w_name}", bufs=1) as wp, \
                 tc.tile_pool(name=f"ps_{w_name}", bufs=3, space=PSUM) as pp:
                wtiles = []
                for k in range(FT):
                    wt = wp.tile([128, D], BF, name=f"w_{w_name}_{k}", tag=f"k{k}")
                    nc.sync.dma_start(wt[:], g[w_name][k * 128:(k + 1) * 128, :])
                    wtiles.append(wt)
                for m in range(FT):
                    ps = pp.tile([128, TOK], F32, tag="ps")
                    for k in range(FT):
                        nc.tensor.matmul(ps[:], wtiles[k][:, m * 128:(m + 1) * 128],
                                         attn_tiles[k][:], start=(k == 0), stop=(k == FT - 1))
                    rt = rpool.tile([128, TOK], F32, name=f"r_{w_name}_{m}", tag=f"r{m}")
                    nc.vector.scalar_tensor_tensor(rt[:], ps[:], bias_col(bo_name, m),
                                                   resid_tiles[m][:], ADD, ADD)
                    outs.append(rt)
            return outs

        def layer_norm(r_tiles, g_name, b_name, out_dtype, out_pool, want_bf16):
            with tc.tile_pool(name=f"lnp_{g_name}", bufs=1, space=PSUM) as lnps, \
                 tc.tile_pool(name=f"lnb_{g_name}", bufs=1, space=PSUM) as lnbc, \
                 tc.tile_pool(name=f"lns_{g_name}", bufs=2) as lnsm, \
                 tc.tile_pool(name=f"lnq_{g_name}", bufs=3) as sqp:
                s1 = lnps.tile([1, TOK], F32, tag="s1")
                s2 = lnps.tile([1, TOK], F32, tag="s2")
                for k in range(FT):
                    nc.tensor.matmul(s1[:], ones_f32[:], r_tiles[k][:],
                                     start=(k == 0), stop=(k == FT - 1))
                for k in range(FT):
                    sq = sqp.tile([128, TOK], F32, tag="sq")
                    nc.vector.tensor_mul(sq[:], r_tiles[k][:], r_tiles[k][:])
                    nc.tensor.matmul(s2[:], ones_f32[:], sq[:],
                                     start=(k == 0), stop=(k == FT - 1))
                mean = lnsm.tile([1, TOK], F32, tag="mean")
                nc.vector.tensor_scalar_mul(mean[:], s1[:], 1.0 / D)
                var = lnsm.tile([1, TOK], F32, tag="var")
                # var = s2/D - mean^2  ==  (s2 * 1/D) + (-mean*mean)
                nc.vector.scalar_tensor_tensor(var[:], mean[:], -1.0, mean[:], MULT, MULT)
                nc.vector.scalar_tensor_tensor(var[:], s2[:], 1.0 / D, var[:], MULT, ADD)
                nc.vector.tensor_scalar_add(var[:], var[:], 1e-5)
                std = lnsm.tile([1, TOK], F32, tag="std")
                nc.scalar.activation(std[:], var[:], mybir.ActivationFunctionType.Sqrt)
                rstd = lnsm.tile([1, TOK], F32, tag="rstd")
                nc.vector.reciprocal(rstd[:], std[:])
                mean_b = lnbc.tile([128, TOK], F32, tag="meanb")
                nc.tensor.matmul(mean_b[:], ones_row[:], mean[:], start=True, stop=True)
                rstd_b = lnbc.tile([128, TOK], F32, tag="rstdb")
                nc.tensor.matmul(rstd_b[:], ones_row[:], rstd[:], start=True, stop=True)
                outs, outs_bf = [], []
                for k in range(FT):
                    xn = sqp.tile([128, TOK], F32, tag="xn")
                    nc.vector.tensor_sub(xn[:], r_tiles[k][:], mean_b[:])
                    nc.vector.tensor_mul(xn[:], xn[:], rstd_b[:])
                    ot = out_pool.tile([128, TOK], out_dtype, name=f"ln_{g_name}_{k}",
                                       tag=f"ln_{k}", bufs=1)
                    nc.vector.tensor_scalar(ot[:], xn[:], bias_col(g_name, k),
                                            bias_col(b_name, k), MULT, ADD)
                    outs.append(ot)
                    if want_bf16:
                        ob = out_pool.tile([128, TOK], BF, name=f"lnb_{g_name}_{k}",
                                           tag=f"lnbf_{k}", bufs=1)
                        nc.scalar.activation(ob[:], ot[:], IDENT)
                        outs_bf.append(ob)
            return outs, outs_bf

        # ================= phase 1-3: attention blocks ======================
        with tc.tile_pool(name="kvc", bufs=1) as kvc_pool:
            with tc.tile_pool(name="kvs", bufs=1) as kvs_pool, \
                 tc.tile_pool(name="x0c", bufs=1) as x0c_pool, \
                 tc.tile_pool(name="mask", bufs=1) as mask_pool, \
                 tc.tile_pool(name="r1p", bufs=1) as r1_pool:
                x0c_t = []
                for k in range(FT):
                    t = x0c_pool.tile([128, TOK], F32, name=f"x0c_{k}", tag=f"x0c_{k}")
                    nc.sync.dma_start(t[:], g['x0chunk'][k * 128:(k + 1) * 128, :])
                    x0c_t.append(t)
                maskT_t = None
                if g['maskT'] is not None:
                    maskT_t = []
                    for k in range(KT):
                        t = mask_pool.tile([128, TOK], F32, name=f"mt_{k}", tag=f"mt_{k}")
                        nc.sync.dma_start(t[:], g['maskT'][k * 128:(k + 1) * 128, :])
                        maskT_t.append(t)

                # phase 1: K/V/Q projections (x0 first, release, then enc)
                with tc.tile_pool(name="acts_x0", bufs=1) as actp:
                    x0_t = []
                    for k in range(FT):
                        t = actp.tile([128, S], BF, name=f"x0_{k}", tag=f"x0_{k}")
                        nc.sync.dma_start(t[:], g['x0fm'][k * 128:(k + 1) * 128, :])
                        x0_t.append(t)
                    k_self = proj_fm('sWk', x0_t, S, 'sbk', kvs_pool)
                    v_self = proj_tm('sWv', x0_t, free_bias['sbv'], kvs_pool)
                with tc.tile_pool(name="acts_enc", bufs=1) as actp:
                    enc_t = []
                    for k in range(FT):
                        t = actp.tile([128, S], BF, name=f"enc_{k}", tag=f"enc_{k}")
                        nc.sync.dma_start(t[:], g['encfm'][k * 128:(k + 1) * 128, :])
                        enc_t.append(t)
                    k_cross = proj_fm('cWk', enc_t, S, 'cbk', kvc_pool)
                    v_cross = proj_tm('cWv', enc_t, free_bias['cbv'], kvc_pool)
                with tc.tile_pool(name="x0cb", bufs=1) as xcb_pool:
                    x0cb = []
                    for k in range(FT):
                        t = xcb_pool.tile([128, TOK], BF, name=f"x0cb_{k}", tag=f"b{k}")
                        nc.vector.tensor_copy(t[:], x0c_t[k][:])
                        x0cb.append(t)
                    q_self = proj_fm('sWq', x0cb, TOK, 'sbq', kvs_pool, scale=0.125)

                # phase 2: self attention + O-proj + LN1
                with tc.tile_pool(name="at_s", bufs=1) as at_pool_s:
                    attn1 = attention(q_self, k_self, v_self, maskT_t, "s", at_pool_s)
                    r1 = o_proj_residual('sWo', attn1, 'sbo', x0c_t, r1_pool)
                a1, a1b = layer_norm(r1, 'ln1_g', 'ln1_b', F32, a1pool, True)

            # phase 3: cross attention + O-proj + LN2
            with tc.tile_pool(name="qc", bufs=1) as qc_pool, \
                 tc.tile_pool(name="maskc", bufs=1) as maskc_pool, \
                 tc.tile_pool(name="r2p", bufs=1) as r2_pool:
                maskTc_t = None
                if g['maskTc'] is not None:
                    maskTc_t = []
                    for k in range(KT):
                        t = maskc_pool.tile([128, TOK], F32, name=f"mtc_{k}", tag=f"mtc_{k}")
                        nc.sync.dma_start(t[:], g['maskTc'][k * 128:(k + 1) * 128, :])
                        maskTc_t.append(t)
                q_cross = proj_fm('cWq', a1b, TOK, 'cbq', qc_pool, scale=0.125)
                with tc.tile_pool(name="at_c", bufs=1) as at_pool_c:
                    attn2 = attention(q_cross, k_cross, v_cross, maskTc_t, "c", at_pool_c)
                    r2 = o_proj_residual('cWo', attn2, 'cbo', a1, r2_pool)
                a2, a2b = layer_norm(r2, 'ln2_g', 'ln2_b', F32, a2pool, True)

        # ================= phase 4: FFN + LN3 ===============================
        with tc.tile_pool(name="hid", bufs=1) as hpool:
            h_tiles = []
            with tc.tile_pool(name="w_fW1", bufs=1) as wp1, \
                 tc.tile_pool(name="ps_f1", bufs=3, space=PSUM) as pp1:
                fw1 = []
                for k in range(FT):
                    wt = wp1.tile([128, DF], BF, name=f"w_fW1_{k}", tag=f"k{k}")
                    nc.sync.dma_start(wt[:], g['fW1'][k * 128:(k + 1) * 128, :])
                    fw1.append(wt)
                for m in range(HT):
                    ps = pp1.tile([128, TOK], F32, tag="ps")
                    for k in range(FT):
                        nc.tensor.matmul(ps[:], fw1[k][:, m * 128:(m + 1) * 128],
                                         a2b[k][:], start=(k == 0), stop=(k == FT - 1))
                    ht = hpool.tile([128, TOK], BF, name=f"h_{m}", tag=f"h_{m}")
                    nc.scalar.activation(ht[:], ps[:], mybir.ActivationFunctionType.Relu,
                                         bias=fb1_sb[:, m:m + 1])
                    h_tiles.append(ht)
            r3 = []
            with tc.tile_pool(name="w_fW2", bufs=1) as wp2, \
                 tc.tile_pool(name="r3p", bufs=1) as r3_pool, \
                 tc.tile_pool(name="ps_f2", bufs=3, space=PSUM) as pp2:
                fw2 = []
                for kh in range(HT):
                    wt = wp2.tile([128, D], BF, name=f"w_fW2_{kh}", tag=f"k{kh}")
                    nc.sync.dma_start(wt[:], g['fW2'][kh * 128:(kh + 1) * 128, :])
                    fw2.append(wt)
                for m in range(FT):
                    ps = pp2.tile([128, TOK], F32, tag="ps")
                    for kh in range(HT):
                        nc.tensor.matmul(ps[:], fw2[kh][:, m * 128:(m + 1) * 128],
                                         h_tiles[kh][:], start=(kh == 0), stop=(kh == HT - 1))
                    rt = r3_pool.tile([128, TOK], F32, name=f"r_ffn_{m}", tag=f"r{m}")
                    nc.vector.scalar_tensor_tensor(rt[:], ps[:], bias_col('fb2', m),
                                                   a2[m][:], ADD, ADD)
                    r3.append(rt)
                y, _ = layer_norm(r3, 'ln3_g', 'ln3_b', BF, ypool, False)

        # ================= phase 5: AllGather of y ==========================
        for k in range(FT):
            nc.sync.dma_start(g['y_sh'][k * 128:(k + 1) * 128, :], y[k][:])
        nc.gpsimd.collective_compute(
            "AllGather", mybir.AluOpType.bypass,
            replica_groups=[list(range(NC))],
            ins=[g['y_sh'][:]], outs=[g['y_ag'][:]])

        # ================= phase 6: vocab projection ========================
        with tc.tile_pool(name="yg", bufs=1) as ygp, \
             tc.tile_pool(name="wout", bufs=2) as woutp, \
             tc.tile_pool(name="vout", bufs=4) as vos, \
             tc.tile_pool(name="vps", bufs=4, space=PSUM) as vps, \
             tc.tile_pool(name="bps", bufs=1, space=PSUM) as bps:
            yg = []
            for r in range(NC):
                row = []
                for k in range(FT):
                    t = ygp.tile([128, TOK], BF, name=f"yg_{r}_{k}", tag=f"yg_{r}_{k}")
                    nc.sync.dma_start(t[:], g['y_ag'][r, k * 128:(k + 1) * 128, :])
                    row.append(t)
                yg.append(row)
            for n in range(VN):
                wn = []
                for k in range(FT):
                    wt = woutp.tile([128, VC], BF, name=f"wout_{n}_{k}", tag=f"wk{k}")
                    nc.sync.dma_start(wt[:], g['Wout'][k * 128:(k + 1) * 128, n * VC:(n + 1) * VC])
                    wn.append(wt)
                brow = woutp.tile([1, VC], F32, name=f"brow_{n}", tag="brow")
                nc.sync.dma_start(brow[:], g['bout_row'][0:1, n * VC:(n + 1) * VC])
                bp = bps.tile([128, VC], F32, tag="bb")
                nc.tensor.matmul(bp[:], ones_row[:], brow[:], start=True, stop=True)
                bout_b = woutp.tile([128, VC], F32, name=f"boutb_{n}", tag="boutb")
                nc.vector.tensor_copy(bout_b[:], bp[:])
                for m in range(NC * TOK // 128):
                    r, half = m // 2, m % 2
                    ps = vps.tile([128, VC], F32, tag="ps")
                    for k in range(FT):
                        nc.tensor.matmul(ps[:], yg[r][k][:, half * 128:(half + 1) * 128],
                                         wn[k][:], start=(k == 0), stop=(k == FT - 1))
                    ot = vos.tile([128, VC], F32, tag="vo")
                    nc.vector.scalar_tensor_tensor(ot[:], ps[:], 1.0, bout_b[:], MULT, ADD)
                    nc.sync.dma_start(g['out'][m * 128:(m + 1) * 128, n * VC:(n + 1) * VC], ot[:])


def host_prep(inputs):
    x0 = np.asarray(inputs['dec_input'], np.float32) + positional_encoding(S, D)[None]
    enc = np.asarray(inputs['enc_input'], np.float32)
    mask_self = np.asarray(inputs['masked_attention_mask'], np.float32)[0, 0]
    mask_cross = np.asarray(inputs['cross_attention_mask'], np.float32)[0, 0]
    self_adds = bool(np.any(mask_self != 0.0))
    cross_adds = bool(np.any(mask_cross != 0.0))
    li = L - 1
    Wl = {}
    for p in ['sWq', 'sWk', 'sWv', 'sWo', 'cWq', 'cWk', 'cWv', 'cWo', 'fW1', 'fW2']:
        Wl[p] = np.ascontiguousarray(np.asarray(inputs[p], np.float32)[li]).astype(BF16)
    bv = {}
    for p in ['sbq', 'sbk', 'sbv', 'sbo', 'cbq', 'cbk', 'cbv', 'cbo',
              'ln1_g', 'ln1_b', 'ln2_g', 'ln2_b', 'ln3_g', 'ln3_b', 'fb1', 'fb2']:
        bv[p] = np.asarray(inputs[p], np.float32)[li]
    Wout = np.asarray(inputs['Wout'], np.float32)
    bout = np.asarray(inputs['bout'], np.float32)

    def pp(v):  # [1024] -> [128, 8] partition-major
        return np.ascontiguousarray(v.reshape(-1, 128).T)

    bias_cols = []
    for name in BIAS_NAMES:
        src = {'sbq': bv['sbq'] * 0.125, 'cbq': bv['cbq'] * 0.125}.get(name, bv.get(name))
        bias_cols.append(pp(src))
    biases_pp = np.ascontiguousarray(np.concatenate(bias_cols, axis=1), np.float32)
    fb1_pp = np.ascontiguousarray(bv['fb1'].reshape(HT, 128).T, np.float32)

    in_maps = []
    for core in range(NC):
        b, c = core // 4, core % 4
        q0 = c * TOK
        m = {
            'x0fm': np.ascontiguousarray(x0[b].T).astype(BF16),
            'encfm': np.ascontiguousarray(enc[b].T).astype(BF16),
            'x0chunk': np.ascontiguousarray(x0[b, q0:q0 + TOK].T, np.float32),
            'biases': biases_pp, 'fb1': fb1_pp,
            'sbv_row': np.ascontiguousarray(bv['sbv'][None, :], np.float32),
            'cbv_row': np.ascontiguousarray(bv['cbv'][None, :], np.float32),
            'bout_row': np.ascontiguousarray(bout[None, core * VS:(core + 1) * VS], np.float32),
            'Wout': np.ascontiguousarray(Wout[:, core * VS:(core + 1) * VS]).astype(BF16),
        }
        m.update(Wl)
        if self_adds:
            m['maskT'] = np.ascontiguousarray(mask_self[q0:q0 + TOK, :].T, np.float32)
        if cross_adds:
            m['maskTc'] = np.ascontiguousarray(mask_cross[q0:q0 + TOK, :].T, np.float32)
        in_maps.append(m)
    return in_maps, self_adds, cross_adds


_CACHE = {}


def _get_program(self_adds, cross_adds):
    key = (self_adds, cross_adds)
    if key not in _CACHE:
        _CACHE[key] = build_program(self_adds, cross_adds)
    return _CACHE[key]


def kernel(**inputs):
    in_maps, self_adds, cross_adds = host_prep(inputs)
    nc = _get_program(self_adds, cross_adds)
    res = run_bass_kernel_spmd(nc, in_maps, core_ids=list(range(NC)))
    shards = [res.results[r]["out"] for r in range(NC)]
    full = np.concatenate(shards, axis=1)          # [2048, V]
    return np.ascontiguousarray(full.reshape(B, S, V), np.float32)


# revision 41
# speedup vs baseline: 1.0059x; 1.0059x over previous
"""Trainium2 Bass kernel for nn_Decoder_3539053052044.

Structure (validated against the reference in numpy first):
- The reference decoder has a preserved bug: every layer consumes the ORIGINAL
  x0, so only the LAST layer's output survives. We compute layer L-1 only.
- Sequence-parallel: 8 cores x 256 tokens (core r -> batch r//4, chunk r%4).
  Each core computes the full last layer for its 256 tokens (K/V projections
  for its whole batch are computed locally), then one AllGather of y (bf16,
  0.5MB/rank), then a vocab-sharded projection (each core: all 2048 tokens x
  its 4000 vocab columns).
- Activations are feature-major [D on partitions, tokens free] so every linear
  layer uses the stored [D_in, D_out] weights directly as lhsT.
- Softmax is max-free (scores are O(1) for this model; exp(-1e9)=0 handles
  masking) and computed directly transposed, scoresT[k,q], so no transposes
  are needed; the per-(head,q) 1/sum is applied after the AV matmul via a
  K=1-matmul partition broadcast.
- LayerNorm runs feature-major via ones-matmul partition reductions.
- bf16 matmul inputs, fp32 accumulation (measured rel err ~3e-3 in the model).
"""

import numpy as np
import ml_dtypes

import concourse.bass as bass
import concourse.bacc as bacc
import concourse.tile as tile
from concourse import mybir
from concourse.bass_utils import run_bass_kernel_spmd
from concourse.vector_clock import ScopedClock, VectorClock

BF16 = ml_dtypes.bfloat16
F32 = mybir.dt.float32
BF = mybir.dt.bfloat16
PSUM = bass.MemorySpace.PSUM

B, S, D, H, L, V, DF = 2, 1024, 1024, 16, 4, 32000, 4096
DH = D // H              # 64
NC = 8                   # cores
TOK = B * S // NC        # 256 tokens per core
VS = V // NC             # 4000 vocab cols per core
KT = S // 128            # 8 k tiles
FT = D // 128            # 8 feature tiles
HT = DF // 128           # 32 hidden tiles
VN = 8                   # vocab n-chunks
VC = VS // VN            # 500 cols per chunk
ADD = mybir.AluOpType.add
MULT = mybir.AluOpType.mult
IDENT = mybir.ActivationFunctionType.Identity

_PATCHED = False


def _patch_tile_drain():
    """This neuronxcc build rejects a Drain carrying >1 sem wait. Split the
    Tile tail drain into one Drain per busy proc, each with a single wait."""
    global _PATCHED
    if _PATCHED:
        return
    _PATCHED = True

    def _drain_and_barrier_split(self, tick_clock, wait_clock):
        gc = tick_clock.global_clock
        n = len(gc)
        for p in range(n):
            if gc[p] > 0:
                vc = VectorClock([gc[q] if q == p else 0 for q in range(n)])
                d = self.nc.sync.drain()
                wait_clock.add_sem_waits(d.ins, ScopedClock({None: vc}))
        self.nc.sync.drain()
        self.nc.all_engine_barrier()
        assert self.sems is not None
        popped = self.nc._tile_sem_poison_stack.pop()
        assert popped is self._sem_poison
        self.nc.clear_and_free_semaphores(list(self.sems.allocated().values()))
        self.nc.all_engine_barrier()

    tile.TileContext._drain_and_barrier = _drain_and_barrier_split


def positional_encoding(seq_len, d_model, n=10000.0):
    i = np.arange(seq_len, dtype=np.float32)[:, None]
    d = np.arange(d_model)
    denom = np.power(n, (2 * (d // 2)).astype(np.float32) / d_model)
    ang = i / denom
    return np.where(d % 2 == 0, np.sin(ang), np.cos(ang)).astype(np.float32)


BIAS_NAMES = ['sbk', 'sbq', 'sbo', 'cbk', 'cbq', 'cbo', 'fb2',
              'ln1_g', 'ln1_b', 'ln2_g', 'ln2_b', 'ln3_g', 'ln3_b']


def build_program(self_mask_adds: bool, cross_mask_adds: bool, zero_free_biases: bool = False,
                  stop_phase: int = 99):
    _patch_tile_drain()
    nc = bacc.Bacc()

    g = {}  # dram handles
    g['x0fm'] = nc.declare_dram_parameter("x0fm", [D, S], BF, isOutput=False)
    g['encfm'] = nc.declare_dram_parameter("encfm", [D, S], BF, isOutput=False)
    g['x0chunk'] = nc.declare_dram_parameter("x0chunk", [D, TOK], F32, isOutput=False)
    for w in ['sWq', 'sWk', 'sWv', 'sWo', 'cWq', 'cWk', 'cWv', 'cWo']:
        g[w] = nc.declare_dram_parameter(w, [D, D], BF, isOutput=False)
    g['fW1'] = nc.declare_dram_parameter("fW1", [D, DF], BF, isOutput=False)
    g['fW2'] = nc.declare_dram_parameter("fW2", [DF, D], BF, isOutput=False)
    g['Wout'] = nc.declare_dram_parameter("Wout", [D, VS], BF, isOutput=False)
    g['biases'] = nc.declare_dram_parameter("biases", [128, 8 * len(BIAS_NAMES)], F32, isOutput=False)
    g['fb1'] = nc.declare_dram_parameter("fb1", [128, HT], F32, isOutput=False)
    g['sbv_row'] = nc.declare_dram_parameter("sbv_row", [1, D], F32, isOutput=False)
    g['cbv_row'] = nc.declare_dram_parameter("cbv_row", [1, D], F32, isOutput=False)
    g['bout_row'] = nc.declare_dram_parameter("bout_row", [1, VS], F32, isOutput=False)
    g['maskT'] = nc.declare_dram_parameter("maskT", [S, 2 * TOK], F32, isOutput=False) if self_mask_adds else None
    g['maskTc'] = nc.declare_dram_parameter("maskTc", [S, 2 * TOK], F32, isOutput=False) if cross_mask_adds else None
    g['out'] = nc.declare_dram_parameter("out", [NC * TOK, VS], F32, isOutput=True)
    g['y_sh'] = nc.dram_tensor("y_sh", [D, TOK], BF)
    g['y_ag'] = nc.dram_tensor("y_ag", [NC, D, TOK], BF, addr_space="Shared")

    with tile.TileContext(nc) as tc:
        _emit(nc, tc, g, zero_free_biases, stop_phase)
    nc.compile()
    return nc


class _StopEmit(Exception):
    pass


def _emit(nc, tc, g, zero_free_biases, stop_phase=99):
    try:
        _emit_inner(nc, tc, g, zero_free_biases, stop_phase)
    except _StopEmit:
        pass


def _emit_inner(nc, tc, g, zero_free_biases, stop_phase):
    def phase_gate(p):
        if stop_phase < p:
            raise _StopEmit()
    from contextlib import ExitStack
    ctx = ExitStack()
    with ctx:
        # ---------- whole-kernel constants / small tensors ------------------
        const = ctx.enter_context(tc.tile_pool(name="const", bufs=1))
        ones_bf = const.tile([128, 1], BF, name="ones_bf", tag="c0")
        nc.gpsimd.memset(ones_bf[:], 1.0)
        ones_f32 = const.tile([128, 1], F32, name="ones_f32", tag="c1")
        nc.gpsimd.memset(ones_f32[:], 1.0)
        ones_row = const.tile([1, 128], F32, name="ones_row", tag="c2")
        nc.gpsimd.memset(ones_row[:], 1.0)
        bias_sb = const.tile([128, 8 * len(BIAS_NAMES)], F32, name="bias_sb", tag="c3")
        nc.sync.dma_start(bias_sb[:], g['biases'][:])
        fb1_sb = const.tile([128, HT], F32, name="fb1_sb", tag="c4")
        nc.sync.dma_start(fb1_sb[:], g['fb1'][:])
        def bias_col(name, f):
            i = BIAS_NAMES.index(name)
            return bias_sb[:, i * 8 + f:i * 8 + f + 1]

        # free-axis bias broadcast tiles [128, D] for sbv / cbv (skipped when
        # the host observed all-zero free-axis biases)
        free_bias = {'sbv': None, 'cbv': None}
        if not zero_free_biases:
            with tc.tile_pool(name="bbc_ps", bufs=1, space=PSUM) as bps, \
                 tc.tile_pool(name="bbc_row", bufs=2) as brow:
                for bi, bname in enumerate(['sbv', 'cbv']):
                    t = const.tile([128, D], F32, name=f"{bname}_b", tag=f"fb{bi}")
                    rsb = brow.tile([1, D], F32, tag="row")
                    nc.sync.dma_start(rsb[:], g[f'{bname}_row'][:])
                    for half in range(2):
                        ps = bps.tile([128, 512], F32, tag="bc")
                        nc.tensor.matmul(ps[:], ones_row[:],
                                         rsb[0:1, half * 512:(half + 1) * 512],
                                         start=True, stop=True)
                        nc.vector.tensor_copy(t[:, half * 512:(half + 1) * 512], ps[:])
                    free_bias[bname] = t

        def copy_out(ot, ps, idx):
            """psum->sbuf copy alternating DVE/ACT to balance engines"""
            if idx % 2 == 0:
                nc.vector.tensor_copy(ot, ps)
            else:
                nc.scalar.activation(ot, ps, IDENT)

        def load_w(w_name, pool, kt_n, cols, parts=1):
            """Load [kt_n*128, cols] weight as `parts` batched tiles.
            Returns accessor: wslice(k, c0, c1) -> [128, c1-c0] lhsT/rhs AP."""
            per = kt_n // parts
            tiles = []
            for pi in range(parts):
                wt = pool.tile([128, per, cols], BF, name=f"w_{w_name}_{pi}", tag=f"w{pi}")
                nc.sync.dma_start(
                    wt[:], g[w_name].rearrange("(a p) d -> p a d", p=128)[:, pi * per:(pi + 1) * per, :])
                tiles.append(wt)

            def wslice(k, c0, c1):
                return tiles[k // per][:, k % per, c0:c1]
            return wslice

        # LN outputs (outer lifetime)
        a1pool = ctx.enter_context(tc.tile_pool(name="a1", bufs=FT))
        a2pool = ctx.enter_context(tc.tile_pool(name="a2", bufs=FT))
        ypool = ctx.enter_context(tc.tile_pool(name="y", bufs=FT))

        # ---------- helpers -------------------------------------------------
        def proj_fm(w_name, act_tiles, n_tok, bias_name, out_pool, scale=None):
            """feature-major out tiles [FT x [128, n_tok]] bf16 = W.T @ act + b"""
            outs = []
            nsub = (n_tok + 511) // 512
            with tc.tile_pool(name=f"w_{w_name}", bufs=1) as wp, \
                 tc.tile_pool(name=f"ps_{w_name}", bufs=4, space=PSUM) as pp:
                w = load_w(w_name, wp, FT, D, parts=2)
                for m in range(FT):
                    ot = out_pool.tile([128, n_tok], BF, name=f"o_{w_name}_{m}",
                                       tag=f"o_{w_name}_{m}", bufs=1)
                    for ns in range(nsub):
                        c0, c1 = ns * 512, min((ns + 1) * 512, n_tok)
                        ps = pp.tile([128, c1 - c0], F32, tag="ps")
                        for k in range(FT):
                            nc.tensor.matmul(ps[:], w(k, m * 128, (m + 1) * 128),
                                             act_tiles[k][:, c0:c1],
                                             start=(k == 0), stop=(k == FT - 1))
                        if scale is not None:
                            nc.vector.tensor_scalar(ot[:, c0:c1], ps[:], scale,
                                                    bias_col(bias_name, m), MULT, ADD)
                        else:
                            nc.scalar.activation(ot[:, c0:c1], ps[:], IDENT,
                                                 bias=bias_col(bias_name, m))
                    outs.append(ot)
            return outs

        def proj_tm(w_name, act_tiles, bias_bcast, out_pool):
            """token-major out tiles [KT x [128, D]] bf16 = act.T @ W + b"""
            outs = []
            with tc.tile_pool(name=f"w_{w_name}", bufs=1) as wp, \
                 tc.tile_pool(name=f"ps_{w_name}", bufs=4, space=PSUM) as pp:
                w = load_w(w_name, wp, FT, D, parts=2)
                for m in range(KT):
                    ot = out_pool.tile([128, D], BF, name=f"o_{w_name}_{m}",
                                       tag=f"o_{w_name}_{m}", bufs=1)
                    for ns in range(2):
                        c0, c1 = ns * 512, (ns + 1) * 512
                        ps = pp.tile([128, 512], F32, tag="ps")
                        for k in range(FT):
                            nc.tensor.matmul(ps[:], act_tiles[k][:, m * 128:(m + 1) * 128],
                                             w(k, c0, c1),
                                             start=(k == 0), stop=(k == FT - 1))
                        if bias_bcast is None:
                            copy_out(ot[:, c0:c1], ps[:], m * 2 + ns)
                        else:
                            nc.vector.scalar_tensor_tensor(ot[:, c0:c1], ps[:], 1.0,
                                                           bias_bcast[:, c0:c1], MULT, ADD)
                    outs.append(ot)
            return outs

        def attention(q_pairs, k_tiles, v_tiles, mask_tiles, tag, out_pool):
            """q_pairs feature-major [FT x [128,TOK]] bf16; k_tiles [FT x [128,S]];
            v_tiles token-major [KT x [128,D]]; mask_tiles doubled [KT x [128,2*TOK]].
            Both heads of a feature pair are processed together: one [128,2*TOK]
            scoresT psum per k-tile -> one mask add -> one exp; AV packs the two
            heads into one [128,TOK] psum via column groups."""
            outs = []
            with tc.tile_pool(name=f"exp_{tag}", bufs=4) as epool, \
                 tc.tile_pool(name=f"asm_{tag}", bufs=4) as spool, \
                 tc.tile_pool(name=f"sT_{tag}", bufs=2, space=PSUM) as sps, \
                 tc.tile_pool(name=f"sum_{tag}", bufs=2, space=PSUM) as sums, \
                 tc.tile_pool(name=f"av_{tag}", bufs=2, space=PSUM) as avs:
                for hp in range(FT):
                    at = out_pool.tile([128, TOK], BF, name=f"at_{tag}_{hp}",
                                       tag=f"at_{hp}", bufs=1)
                    sum0 = sums.tile([1, TOK], F32, tag="sum")
                    sum1 = sums.tile([1, TOK], F32, tag="sum")
                    av0 = avs.tile([64, TOK], F32, tag="av")
                    av1 = avs.tile([64, TOK], F32, tag="av")
                    for kt in range(KT):
                        # two heads' scoresT in one 2-bank psum tile (one matmul
                        # group per bank); mask-add + exp fused via strided APs
                        sTp = sps.tile([128, 4 * TOK], F32, tag="sT")
                        for hh in range(2):
                            po = hh * 64
                            nc.tensor.matmul(
                                sTp[:, hh * 2 * TOK:hh * 2 * TOK + TOK],
                                k_tiles[hp][po:po + 64, kt * 128:(kt + 1) * 128],
                                q_pairs[hp][po:po + 64, :], start=True, stop=True)
                        sview = sTp.rearrange("p (b c) -> p b c", c=2 * TOK)[:, :, 0:TOK]
                        if mask_tiles is not None:
                            mview = mask_tiles[kt].rearrange("p (b c) -> p b c", c=TOK)
                            nc.vector.tensor_add(sview, sview, mview)
                        ex = epool.tile([128, 2 * TOK], BF, tag="exp")
                        exv = ex.rearrange("p (b c) -> p b c", c=TOK)
                        nc.scalar.activation(exv, sview, mybir.ActivationFunctionType.Exp)
                        for hh, sm, av in ((0, sum0, av0), (1, sum1, av1)):
                            h = 2 * hp + hh
                            nc.tensor.matmul(sm[:], ones_bf[:], ex[:, hh * TOK:(hh + 1) * TOK],
                                             start=(kt == 0), stop=(kt == KT - 1))
                            nc.tensor.matmul(av[:],
                                             v_tiles[kt][:, h * 64:(h + 1) * 64],
                                             ex[:, hh * TOK:(hh + 1) * TOK],
                                             start=(kt == 0), stop=(kt == KT - 1))
                    for hh, sm, av in ((0, sum0, av0), (1, sum1, av1)):
                        rec = spool.tile([1, TOK], F32, tag="recip")
                        nc.vector.reciprocal(rec[:], sm[:])
                        rb = sps.tile([64, TOK], F32, tag="sT")
                        nc.tensor.matmul(rb[:], ones_row[0:1, 0:64], rec[:],
                                         start=True, stop=True)
                        rb_sb = spool.tile([64, TOK], F32, tag="rb_sb")
                        nc.vector.tensor_copy(rb_sb[:], rb[:])
                        nc.vector.tensor_mul(at[hh * 64:hh * 64 + 64, :], av[:], rb_sb[:])
                    outs.append(at)
            return outs

        def o_proj_residual(w_name, attn_tiles, bo_name, resid_tiles, rpool):
            outs = []
            with tc.tile_pool(name=f"w_{w_name}", bufs=1) as wp, \
                 tc.tile_pool(name=f"ps_{w_name}", bufs=4, space=PSUM) as pp:
                w = load_w(w_name, wp, FT, D, parts=2)
                for m in range(FT):
                    ps = pp.tile([128, TOK], F32, tag="ps")
                    for k in range(FT):
                        nc.tensor.matmul(ps[:], w(k, m * 128, (m + 1) * 128),
                                         attn_tiles[k][:], start=(k == 0), stop=(k == FT - 1))
                    rt = rpool.tile([128, TOK], F32, name=f"r_{w_name}_{m}", tag=f"r{m}")
                    nc.vector.scalar_tensor_tensor(rt[:], ps[:], bias_col(bo_name, m),
                                                   resid_tiles[m][:], ADD, ADD)
                    outs.append(rt)
            return outs

        def layer_norm(r_tiles, g_name, b_name, out_dtype, out_pool, want_bf16):
            with tc.tile_pool(name=f"lnp_{g_name}", bufs=1, space=PSUM) as lnps, \
                 tc.tile_pool(name=f"lnb_{g_name}", bufs=1, space=PSUM) as lnbc, \
                 tc.tile_pool(name=f"lns_{g_name}", bufs=2) as lnsm, \
                 tc.tile_pool(name=f"lnq_{g_name}", bufs=3) as sqp:
                s1 = lnps.tile([1, TOK], F32, tag="s1")
                s2 = lnps.tile([1, TOK], F32, tag="s2")
                for k in range(FT):
                    nc.tensor.matmul(s1[:], ones_f32[:], r_tiles[k][:],
                                     start=(k == 0), stop=(k == FT - 1))
                for k in range(FT):
                    sq = sqp.tile([128, TOK], F32, tag="sq")
                    nc.vector.tensor_mul(sq[:], r_tiles[k][:], r_tiles[k][:])
                    nc.tensor.matmul(s2[:], ones_f32[:], sq[:],
                                     start=(k == 0), stop=(k == FT - 1))
                mean = lnsm.tile([1, TOK], F32, tag="mean")
                nc.vector.tensor_scalar_mul(mean[:], s1[:], 1.0 / D)
                var = lnsm.tile([1, TOK], F32, tag="var")
                # var = s2/D - mean^2  ==  (s2 * 1/D) + (-mean*mean)
                nc.vector.scalar_tensor_tensor(var[:], mean[:], -1.0, mean[:], MULT, MULT)
                nc.vector.scalar_tensor_tensor(var[:], s2[:], 1.0 / D, var[:], MULT, ADD)
                nc.vector.tensor_scalar_add(var[:], var[:], 1e-5)
                std = lnsm.tile([1, TOK], F32, tag="std")
                nc.scalar.activation(std[:], var[:], mybir.ActivationFunctionType.Sqrt)
                rstd = lnsm.tile([1, TOK], F32, tag="rstd")
                nc.vector.reciprocal(rstd[:], std[:])
                mean_b = lnbc.tile([128, TOK], F32, tag="meanb")
                nc.tensor.matmul(mean_b[:], ones_row[:], mean[:], start=True, stop=True)
                rstd_b = lnbc.tile([128, TOK], F32, tag="rstdb")
                nc.tensor.matmul(rstd_b[:], ones_row[:], rstd[:], start=True, stop=True)
                outs, outs_bf = [], []
                for k in range(FT):
                    xn = sqp.tile([128, TOK], F32, tag="xn")
                    nc.vector.tensor_sub(xn[:], r_tiles[k][:], mean_b[:])
                    nc.vector.tensor_mul(xn[:], xn[:], rstd_b[:])
                    ot = out_pool.tile([128, TOK], out_dtype, name=f"ln_{g_name}_{k}",
                                       tag=f"ln_{k}", bufs=1)
                    nc.vector.tensor_scalar(ot[:], xn[:], bias_col(g_name, k),
                                            bias_col(b_name, k), MULT, ADD)
                    outs.append(ot)
                    if want_bf16:
                        ob = out_pool.tile([128, TOK], BF, name=f"lnb_{g_name}_{k}",
                                           tag=f"lnbf_{k}", bufs=1)
                        nc.scalar.activation(ob[:], ot[:], IDENT)
                        outs_bf.append(ob)
            return outs, outs_bf

        # ================= phase 1-3: attention blocks ======================
        with tc.tile_pool(name="kvc", bufs=1) as kvc_pool:
            with tc.tile_pool(name="kvs", bufs=1) as kvs_pool, \
                 tc.tile_pool(name="x0c", bufs=1) as x0c_pool, \
                 tc.tile_pool(name="mask", bufs=1) as mask_pool, \
                 tc.tile_pool(name="r1p", bufs=1) as r1_pool:
                x0c_b = x0c_pool.tile([128, FT, TOK], F32, name="x0c_b", tag="x0c")
                nc.sync.dma_start(x0c_b[:], g['x0chunk'].rearrange("(a p) t -> p a t", p=128)[:])
                x0c_t = [x0c_b[:, k, :] for k in range(FT)]
                maskT_t = None
                if g['maskT'] is not None:
                    mt_b = mask_pool.tile([128, KT, 2 * TOK], F32, name="mt_b", tag="mt")
                    nc.sync.dma_start(mt_b[:], g['maskT'].rearrange("(a p) t -> p a t", p=128)[:])
                    maskT_t = [mt_b[:, k, :] for k in range(KT)]

                # phase 1: K/V/Q projections (x0 first, release, then enc)
                with tc.tile_pool(name="acts_x0", bufs=1) as actp:
                    x0_b = actp.tile([128, FT, S], BF, name="x0_b", tag="x0")
                    x0r = g['x0fm'].rearrange("(a p) t -> p a t", p=128)
                    nc.sync.dma_start(x0_b[:, 0:4, :], x0r[:, 0:4, :])
                    nc.sync.dma_start(x0_b[:, 4:8, :], x0r[:, 4:8, :])
                    x0_t = [x0_b[:, k, :] for k in range(FT)]
                    k_self = proj_fm('sWk', x0_t, S, 'sbk', kvs_pool)
                    v_self = proj_tm('sWv', x0_t, free_bias['sbv'], kvs_pool)
                with tc.tile_pool(name="x0cb", bufs=1) as xcb_pool:
                    x0cb = []
                    for k in range(FT):
                        t = xcb_pool.tile([128, TOK], BF, name=f"x0cb_{k}", tag=f"b{k}")
                        nc.vector.tensor_copy(t[:], x0c_t[k][:])
                        x0cb.append(t)
                    q_self = proj_fm('sWq', x0cb, TOK, 'sbq', kvs_pool, scale=0.125)
                with tc.tile_pool(name="acts_enc", bufs=1) as actp:
                    enc_b = actp.tile([128, FT, S], BF, name="enc_b", tag="enc")
                    encr = g['encfm'].rearrange("(a p) t -> p a t", p=128)
                    nc.sync.dma_start(enc_b[:, 0:4, :], encr[:, 0:4, :])
                    nc.sync.dma_start(enc_b[:, 4:8, :], encr[:, 4:8, :])
                    enc_t = [enc_b[:, k, :] for k in range(FT)]
                    k_cross = proj_fm('cWk', enc_t, S, 'cbk', kvc_pool)
                    v_cross = proj_tm('cWv', enc_t, free_bias['cbv'], kvc_pool)

                phase_gate(1)
                # phase 2: self attention + O-proj + LN1
                with tc.tile_pool(name="at_s", bufs=1) as at_pool_s:
                    attn1 = attention(q_self, k_self, v_self, maskT_t, "s", at_pool_s)
                    r1 = o_proj_residual('sWo', attn1, 'sbo', x0c_t, r1_pool)
                a1, a1b = layer_norm(r1, 'ln1_g', 'ln1_b', F32, a1pool, True)

            phase_gate(2)
            # phase 3: cross attention + O-proj + LN2
            with tc.tile_pool(name="qc", bufs=1) as qc_pool, \
                 tc.tile_pool(name="maskc", bufs=1) as maskc_pool, \
                 tc.tile_pool(name="r2p", bufs=1) as r2_pool:
                maskTc_t = None
                if g['maskTc'] is not None:
                    mtc_b = maskc_pool.tile([128, KT, 2 * TOK], F32, name="mtc_b", tag="mtc")
                    nc.sync.dma_start(mtc_b[:], g['maskTc'].rearrange("(a p) t -> p a t", p=128)[:])
                    maskTc_t = [mtc_b[:, k, :] for k in range(KT)]
                q_cross = proj_fm('cWq', a1b, TOK, 'cbq', qc_pool, scale=0.125)
                with tc.tile_pool(name="at_c", bufs=1) as at_pool_c:
                    attn2 = attention(q_cross, k_cross, v_cross, maskTc_t, "c", at_pool_c)
                    r2 = o_proj_residual('cWo', attn2, 'cbo', a1, r2_pool)
                a2, a2b = layer_norm(r2, 'ln2_g', 'ln2_b', F32, a2pool, True)

        phase_gate(3)
        # ================= phase 4: FFN + LN3 ===============================
        with tc.tile_pool(name="hid", bufs=1) as hpool:
            h_tiles = []
            with tc.tile_pool(name="w_fW1", bufs=1) as wp1, \
                 tc.tile_pool(name="ps_f1", bufs=3, space=PSUM) as pp1:
                w1 = load_w('fW1', wp1, FT, DF, parts=4)
                for m in range(HT):
                    ps = pp1.tile([128, TOK], F32, tag="ps")
                    for k in range(FT):
                        nc.tensor.matmul(ps[:], w1(k, m * 128, (m + 1) * 128),
                                         a2b[k][:], start=(k == 0), stop=(k == FT - 1))
                    ht = hpool.tile([128, TOK], BF, name=f"h_{m}", tag=f"h_{m}")
                    nc.scalar.activation(ht[:], ps[:], mybir.ActivationFunctionType.Relu,
                                         bias=fb1_sb[:, m:m + 1])
                    h_tiles.append(ht)
            r3 = []
            with tc.tile_pool(name="w_fW2", bufs=1) as wp2, \
                 tc.tile_pool(name="r3p", bufs=1) as r3_pool, \
                 tc.tile_pool(name="ps_f2", bufs=3, space=PSUM) as pp2:
                w2 = load_w('fW2', wp2, HT, D, parts=4)
                for m in range(FT):
                    ps = pp2.tile([128, TOK], F32, tag="ps")
                    for kh in range(HT):
                        nc.tensor.matmul(ps[:], w2(kh, m * 128, (m + 1) * 128),
                                         h_tiles[kh][:], start=(kh == 0), stop=(kh == HT - 1))
                    rt = r3_pool.tile([128, TOK], F32, name=f"r_ffn_{m}", tag=f"r{m}")
                    nc.vector.scalar_tensor_tensor(rt[:], ps[:], bias_col('fb2', m),
                                                   a2[m][:], ADD, ADD)
                    r3.append(rt)
                y, _ = layer_norm(r3, 'ln3_g', 'ln3_b', BF, ypool, False)

        phase_gate(4)
        # ================= phase 5: AllGather of y ==========================
        for k in range(FT):
            nc.sync.dma_start(g['y_sh'][k * 128:(k + 1) * 128, :], y[k][:])
        nc.gpsimd.collective_compute(
            "AllGather", mybir.AluOpType.bypass,
            replica_groups=[list(range(NC))],
            ins=[g['y_sh'][:]], outs=[g['y_ag'][:]])

        phase_gate(5)
        # ================= phase 6: vocab projection ========================
        with tc.tile_pool(name="yg", bufs=1) as ygp, \
             tc.tile_pool(name="wout", bufs=1) as woutp, \
             tc.tile_pool(name="vout", bufs=2) as vos, \
             tc.tile_pool(name="vps", bufs=4, space=PSUM) as vps, \
             tc.tile_pool(name="bps", bufs=1, space=PSUM) as bps:
            yg = []
            for r in range(NC):
                t = ygp.tile([128, FT, TOK], BF, name=f"yg_{r}", tag=f"yg_{r}")
                nc.sync.dma_start(t[:], g['y_ag'][r].rearrange("(a p) t -> p a t", p=128)[:])
                yg.append(t)
            wv = load_w('Wout', woutp, FT, VS, parts=4)
            bout_b = None
            if not zero_free_biases:
                brow = woutp.tile([1, VS], F32, name="brow", tag="brow")
                nc.sync.dma_start(brow[:], g['bout_row'][:])
                bout_b = woutp.tile([128, VS], F32, name="boutb", tag="boutb")
                for n in range(VN):
                    bp = bps.tile([128, VC], F32, tag="bb")
                    nc.tensor.matmul(bp[:], ones_row[:], brow[0:1, n * VC:(n + 1) * VC],
                                     start=True, stop=True)
                    nc.vector.tensor_copy(bout_b[:, n * VC:(n + 1) * VC], bp[:])
            for m in range(NC * TOK // 128):
                r, half = m // 2, m % 2
                ot = vos.tile([128, VS], F32, tag="vo")
                for n in range(VN):
                    ps = vps.tile([128, VC], F32, tag="ps")
                    for k in range(FT):
                        nc.tensor.matmul(ps[:], yg[r][:, k, half * 128:(half + 1) * 128],
                                         wv(k, n * VC, (n + 1) * VC),
                                         start=(k == 0), stop=(k == FT - 1))
                    if bout_b is None:
                        copy_out(ot[:, n * VC:(n + 1) * VC], ps[:], n)
                    else:
                        nc.vector.scalar_tensor_tensor(ot[:, n * VC:(n + 1) * VC], ps[:],
                                                       1.0, bout_b[:, n * VC:(n + 1) * VC],
                                                       MULT, ADD)
                nc.sync.dma_start(g['out'][m * 128:(m + 1) * 128, :], ot[:])


def host_prep(inputs):
    x0 = np.asarray(inputs['dec_input'], np.float32) + positional_encoding(S, D)[None]
    enc = np.asarray(inputs['enc_input'], np.float32)
    mask_self = np.asarray(inputs['masked_attention_mask'], np.float32)[0, 0]
    mask_cross = np.asarray(inputs['cross_attention_mask'], np.float32)[0, 0]
    self_adds = bool(np.any(mask_self != 0.0))
    cross_adds = bool(np.any(mask_cross != 0.0))
    li = L - 1
    Wl = {}
    for p in ['sWq', 'sWk', 'sWv', 'sWo', 'cWq', 'cWk', 'cWv', 'cWo', 'fW1', 'fW2']:
        Wl[p] = np.ascontiguousarray(np.asarray(inputs[p], np.float32)[li]).astype(BF16)
    bv = {}
    for p in ['sbq', 'sbk', 'sbv', 'sbo', 'cbq', 'cbk', 'cbv', 'cbo',
              'ln1_g', 'ln1_b', 'ln2_g', 'ln2_b', 'ln3_g', 'ln3_b', 'fb1', 'fb2']:
        bv[p] = np.asarray(inputs[p], np.float32)[li]
    Wout = np.asarray(inputs['Wout'], np.float32)
    bout = np.asarray(inputs['bout'], np.float32)

    def pp(v):  # [1024] -> [128, 8] partition-major
        return np.ascontiguousarray(v.reshape(-1, 128).T)

    bias_cols = []
    for name in BIAS_NAMES:
        src = {'sbq': bv['sbq'] * 0.125, 'cbq': bv['cbq'] * 0.125}.get(name, bv.get(name))
        bias_cols.append(pp(src))
    biases_pp = np.ascontiguousarray(np.concatenate(bias_cols, axis=1), np.float32)
    fb1_pp = np.ascontiguousarray(bv['fb1'].reshape(HT, 128).T, np.float32)

    in_maps = []
    for core in range(NC):
        b, c = core // 4, core % 4
        q0 = c * TOK
        m = {
            'x0fm': np.ascontiguousarray(x0[b].T).astype(BF16),
            'encfm': np.ascontiguousarray(enc[b].T).astype(BF16),
            'x0chunk': np.ascontiguousarray(x0[b, q0:q0 + TOK].T, np.float32),
            'biases': biases_pp, 'fb1': fb1_pp,
            'sbv_row': np.ascontiguousarray(bv['sbv'][None, :], np.float32),
            'cbv_row': np.ascontiguousarray(bv['cbv'][None, :], np.float32),
            'bout_row': np.ascontiguousarray(bout[None, core * VS:(core + 1) * VS], np.float32),
            'Wout': np.ascontiguousarray(Wout[:, core * VS:(core + 1) * VS]).astype(BF16),
        }
        m.update(Wl)
        if self_adds:
            mt = mask_self[q0:q0 + TOK, :].T
            m['maskT'] = np.ascontiguousarray(np.concatenate([mt, mt], axis=1), np.float32)
        if cross_adds:
            mt = mask_cross[q0:q0 + TOK, :].T
            m['maskTc'] = np.ascontiguousarray(np.concatenate([mt, mt], axis=1), np.float32)
        in_maps.append(m)
    zero_free = not (np.any(bv['sbv']) or np.any(bv['cbv']) or np.any(bout))
    return in_maps, self_adds, cross_adds, zero_free


_CACHE = {}


def _get_program(self_adds, cross_adds, zero_free):
    key = (self_adds, cross_adds, zero_free)
    if key not in _CACHE:
        _CACHE[key] = build_program(self_adds, cross_adds, zero_free)
    return _CACHE[key]


def kernel(**inputs):
    in_maps, self_adds, cross_adds, zero_free = host_prep(inputs)
    nc = _get_program(self_adds, cross_adds, zero_free)
    res = run_bass_kernel_spmd(nc, in_maps, core_ids=list(range(NC)))
    shards = [res.results[r]["out"] for r in range(NC)]
    full = np.concatenate(shards, axis=1)          # [2048, V]
    return np.ascontiguousarray(full.reshape(B, S, V), np.float32)
